# revision 2
# baseline (speedup 1.0000x reference)
"""Multi-head attention (B=2, S=2048, EMB=1024, H=16) on 8 Trainium2 cores.

v3 — tensor-parallel over heads: core c owns heads {2c, 2c+1} (a 128-wide
feature slice F_c of the QKV projections and of Wo's rows).  Each core:
  - projects q/k/v for ALL 4096 tokens through its 1024x128 weight slices
  - runs attention for its 2 heads over both batches
  - computes the PARTIAL output projection out_c = att_c @ Wo[:, F_c].T
The 8 partial outputs (bf16) are summed on the host (linear in Wo rows), so
no device collectives are needed and no compute is redundant: per-core FLOPs
drop from ~15 G (query-sharded baseline with 4x-redundant K/V projections)
to ~8.6 G (perfect 1/8 split of total work).

Layouts (all device matmuls contract over the partition dim):
  - inputs staged HOST-side as [8 blocks, 128, 8, 512] so every x-block DMA
    is one 8 KiB-contiguous run per partition (descriptor count, not
    bandwidth, limited an earlier cut)
  - qhT/khT [128 = (headA 64 | headB 64), 4096 tok] bf16, feature-major
  - vhe [128 tok, chunk, head, 65] bf16 token-major with an all-ones 65th
    column so the att matmul accumulates softmax denominators for free
  - scores_T [k, q] per key-chunk via a 2x2 tile_position quad: 4 matmuls
    (2 heads x 2 key-halves, K=64 M=64 N=512) occupy disjoint PE-array
    quadrants and run concurrently (measured 54 ns/MM vs 217 serial)
  - exp: one ACT call per chunk ([128, 1024] over a 2-bank PSUM tile,
    scale=1/8, no max subtraction; |scores|/8 < ~7 so exp stays in range)
  - normalize: copy denom row, gpsimd partition_broadcast, DVE
    reciprocal_approx_fast on [64, 512] (full-width lanes), multiply
  - out partial [tok, 1024] bf16: one DMA per 128-token chunk, issued from
    the gpsimd queue to keep descriptor generation off the sync engine

Pipelining: the score PSUM is double-buffered (scps bufs=2), so the quad of
chunk j+1 fills one 2-bank buffer while ACT exp-drains the other — ACT runs
gapless and the PE never idles long enough for the HAM clock gate to drop
the array to 1.2 GHz (the dominant loss in earlier cuts).  All other PE work
(v-projection of batch 0, every projection of batch 1, the out-projection of
the previous qblock) is queued as <=1.2us filler units, popped one per chunk
into the PE's ~0.5us-per-chunk slack.
"""

from collections import deque
from contextlib import ExitStack

import numpy as np
import ml_dtypes

import concourse.bass as bass  # noqa: F401
import concourse.mybir as mybir
import concourse.tile as tile
from concourse import bacc
from concourse.bass_utils import run_bass_kernel_spmd

BF = mybir.dt.bfloat16
F32 = mybir.dt.float32

EMB = 1024
HEADS = 16
HD = EMB // HEADS          # 64
B, S = 2, 2048
N_CORES = 8
P = 128
NE = EMB // P              # 8 contraction chunks
T = B * S                  # 4096 tokens
NT = T // P                # 32 token chunks
SB = S // P                # 16 key chunks per batch
QB = 512                   # query block (psum bank width)
NQ = S // QB               # 4 qblocks per batch
NBLK = T // QB             # 8 x-blocks
EXPF = mybir.ActivationFunctionType.Exp
SCALE = 1.0 / np.sqrt(HD)  # 0.125


def _build_nc(with_bv: bool, with_bo: bool):
    nc = bacc.Bacc(num_devices=N_CORES)
    dp = nc.declare_dram_parameter
    qT = dp("qT", [NBLK, P, NE, QB], BF, isOutput=False)
    kT = dp("kT", [NBLK, P, NE, QB], BF, isOutput=False)
    vT = dp("vT", [NBLK, P, NE, QB], BF, isOutput=False)
    WqT = dp("WqT", [P, NE, P], BF, isOutput=False)
    WkT = dp("WkT", [P, NE, P], BF, isOutput=False)
    WvT = dp("WvT", [P, NE, P], BF, isOutput=False)
    WoT = dp("WoT", [P, EMB], BF, isOutput=False)
    bqp = dp("bqp", [P, 1], F32, isOutput=False)
    bkp = dp("bkp", [P, 1], F32, isOutput=False)
    bvr = dp("bvr", [1, P], BF, isOutput=False)
    out = dp("out", [T, EMB], BF, isOutput=True)

    with tile.TileContext(nc) as tc, ExitStack() as ctx:
        wpool = ctx.enter_context(tc.tile_pool(name="wts", bufs=1))
        apool = ctx.enter_context(tc.tile_pool(name="acts", bufs=1))
        xpool = ctx.enter_context(tc.tile_pool(name="xin", bufs=6))
        ppool = ctx.enter_context(tc.tile_pool(name="probs", bufs=4))
        dpool = ctx.enter_context(tc.tile_pool(name="dn", bufs=4))
        opool = ctx.enter_context(tc.tile_pool(name="ob", bufs=3))
        genps = ctx.enter_context(tc.tile_pool(name="gen", bufs=2, space="PSUM"))
        scps = ctx.enter_context(tc.tile_pool(name="sc", bufs=2, space="PSUM"))
        attps = ctx.enter_context(tc.tile_pool(name="att", bufs=1, space="PSUM"))

        qhT = apool.tile([P, T], BF, tag="qhT")
        khT = apool.tile([P, T], BF, tag="khT")
        vhe = apool.tile([P, NT, 2, HD + 1], BF, tag="vhe")
        attT = apool.tile([P, T], BF, tag="attT")

        WqT_sb = wpool.tile([P, NE, P], BF, tag="WqT")
        WkT_sb = wpool.tile([P, NE, P], BF, tag="WkT")
        WvT_sb = wpool.tile([P, NE, P], BF, tag="WvT")
        WoT_sb = wpool.tile([P, EMB], BF, tag="WoT")
        bqp_sb = wpool.tile([P, 1], F32, tag="bqp")
        bkp_sb = wpool.tile([P, 1], F32, tag="bkp")
        if with_bv:
            ones_b = wpool.tile([1, P], BF, tag="ones")
            nc.vector.memset(ones_b[:], 1.0)
            bvr_sb = wpool.tile([1, P], BF, tag="bvr")
            nc.sync.dma_start(bvr_sb[:], bvr[:])

        nc.vector.memset(vhe[:, :, :, HD], 1.0)

        # ---- DMA staging: one 8KiB-contiguous-per-partition DMA per block;
        # emitting all up front lets ring-slot WAR deps pace the prefetch ----
        nc.sync.dma_start(WqT_sb[:], WqT[:])
        nc.sync.dma_start(bqp_sb[:], bqp[:])
        nc.sync.dma_start(bkp_sb[:], bkp[:])
        nc.sync.dma_start(WkT_sb[:], WkT[:])
        nc.sync.dma_start(WvT_sb[:], WvT[:])
        xblocks = {}

        def stage_block(name, src, b, nb):
            xb = xpool.tile([P, NE, QB], BF, tag="xin", name=f"x_{name}_{b}_{nb}")
            nc.sync.dma_start(xb[:], src[b * NQ + nb, :, :, :])
            xblocks[(name, b, nb)] = xb

        # order: q-blk0 + all k of b0 (the minimum for the first score
        # quads), then v b0 (attention side), then q b0 rest, Wo, all of b1
        stage_block("q", qT, 0, 0)
        for nb in range(NQ):
            stage_block("k", kT, 0, nb)
        for nb in range(NQ):
            stage_block("v", vT, 0, nb)
        for nb in range(1, NQ):
            stage_block("q", qT, 0, nb)
        nc.sync.dma_start(WoT_sb[:], WoT[:])
        for name, src in (("q", qT), ("k", kT), ("v", vT)):
            for nb in range(NQ):
                stage_block(name, src, 1, nb)

        # ---- emission helpers ----
        def qk_proj_half(dst, W_sb, xname, bias_sb, b, nb, half, ps_box):
            """Half of a 512-token projection block (4 of 8 k-chunks)."""
            t0 = b * S + nb * QB
            xb = xblocks[(xname, b, nb)]
            if half == 0:
                ps_box[0] = genps.tile(
                    [P, QB], F32, tag="gen", name=f"pj_{xname}_{b}_{nb}"
                )
            ps = ps_box[0]
            for kk in range(half * 4, half * 4 + 4):
                nc.tensor.matmul(
                    ps[:],
                    W_sb[:, kk, :],
                    xb[:, kk, :],
                    start=(kk == 0),
                    stop=(kk == NE - 1),
                )
            if half == 1:
                nc.vector.tensor_scalar_add(
                    dst[:, t0 : t0 + QB], ps[:], bias_sb[:, 0:1]
                )

        def qk_proj_unit(dst, W_sb, xname, bias_sb, b, nb):
            box = [None]
            qk_proj_half(dst, W_sb, xname, bias_sb, b, nb, 0, box)
            qk_proj_half(dst, W_sb, xname, bias_sb, b, nb, 1, box)

        def qk_halves(dst, W_sb, xname, bias_sb, b, nb):
            box = [None]
            return [
                (lambda h=h: qk_proj_half(dst, W_sb, xname, bias_sb, b, nb, h, box))
                for h in range(2)
            ]

        def vh_proj_unit(b, m):
            """Project one 128-token chunk of v into vhe (token-major)."""
            mm = b * SB + m
            xb = xblocks[("v", b, m // (QB // P))]
            mi = m % (QB // P)
            ps = genps.tile([P, QB], F32, tag="gen", name=f"pv_{mm}")
            for kk in range(NE):
                nc.tensor.matmul(
                    ps[:, 0:P],
                    xb[:, kk, mi * P : (mi + 1) * P],
                    WvT_sb[:, kk, :],
                    start=(kk == 0),
                    stop=(kk == NE - 1) and not with_bv,
                )
            if with_bv:
                nc.tensor.matmul(
                    ps[:, 0:P], ones_b[:], bvr_sb[:], start=False, stop=True
                )
            nc.vector.tensor_copy(
                vhe[:, mm, :, 0:HD], ps[:, 0:P].rearrange("p (h d) -> p h d", d=HD)
            )

        prio: deque = deque()     # normalize tails + out-proj: no DMA deps
        fillers: deque = deque()  # (min_step, fn): gated on x-block DMA arrival
        last_prio = [-10]

        def fill(step):
            # prio units carry DVE-heavy epilogue work; at most one per 2
            # steps so the DVE never backlogs the gen-psum ring (a backlog
            # stalls the in-order PE queue and gaps the exp pipeline)
            if prio and step - last_prio[0] >= 2:
                last_prio[0] = step
                prio.popleft()()
            elif fillers and fillers[0][0] <= step:
                fillers.popleft()[1]()

        def sc_chunk(b, qi, j):
            """Scores+exp for one key chunk: a 2x2 matmul quad, one ACT call."""
            q0 = b * S + qi * QB
            j0 = (b * SB + j) * P
            sc = scps.tile([P, 2 * QB], F32, tag="sc", name=f"sc_{b}_{qi}_{j}")
            nc.tensor.matmul(
                sc[0:HD, 0:QB],
                khT[0:HD, j0 : j0 + HD],
                qhT[0:HD, q0 : q0 + QB],
                start=True, stop=True,
            )
            nc.tensor.matmul(
                sc[HD:P, 0:QB],
                khT[0:HD, j0 + HD : j0 + P],
                qhT[0:HD, q0 : q0 + QB],
                start=True, stop=True,
            )
            nc.tensor.matmul(
                sc[0:HD, QB:],
                khT[HD:P, j0 : j0 + HD],
                qhT[HD:P, q0 : q0 + QB],
                start=True, stop=True,
            )
            nc.tensor.matmul(
                sc[HD:P, QB:],
                khT[HD:P, j0 + HD : j0 + P],
                qhT[HD:P, q0 : q0 + QB],
                start=True, stop=True,
            )
            pr = ppool.tile([P, 2 * QB], BF, tag="pr", name=f"pr_{b}_{qi}_{j}")
            nc.scalar.activation(pr[:], sc[:], EXPF, scale=SCALE)
            return pr

        def att_chunk(b, j, pr, attAB):
            jb = b * SB + j
            first = j == 0
            last = j == SB - 1
            nc.tensor.matmul(
                attAB[:, 0, :], vhe[:, jb, 0, :], pr[:, 0:QB], start=first, stop=last
            )
            nc.tensor.matmul(
                attAB[:, 1, :], vhe[:, jb, 1, :], pr[:, QB:], start=first, stop=last
            )

        def normalize_block(b, qi, attAB, last=False):
            # copy the accumulators out of PSUM in ONE op: the att psum slot
            # is WAR-waited by the NEXT qblock's first att matmul (in-order
            # PE queue!), so it must free fast, not after the full
            # broadcast/reciprocal/multiply chain
            q0 = b * S + qi * QB
            if last:
                # nothing reuses the att psum slot afterwards: skip the
                # staging copy and read PSUM directly (shorter drain chain)
                au2 = attAB
            else:
                au2 = dpool.tile(
                    [HD + 1, 2, QB], F32, tag="au", name=f"au_{b}_{qi}"
                )
                nc.vector.tensor_copy(au2[:], attAB[:])

            def tail(h):
                d0 = dpool.tile([1, QB], F32, tag=f"d0{h}", name=f"d0_{b}_{qi}_{h}")
                nc.vector.tensor_copy(d0[:], au2[HD : HD + 1, h, :])
                r0 = dpool.tile([1, QB], F32, tag=f"r0{h}", name=f"r0_{b}_{qi}_{h}")
                nc.vector.reciprocal_approx_fast(r0[:], d0[:])
                rb = dpool.tile([HD, QB], F32, tag=f"rb{h}", name=f"rb_{b}_{qi}_{h}")
                nc.gpsimd.partition_broadcast(rb[:], r0[:])
                nc.vector.tensor_mul(
                    attT[h * HD : (h + 1) * HD, q0 : q0 + QB],
                    au2[0:HD, h, :],
                    rb[:],
                )

            if last:
                tail(0)
                tail(1)
            else:
                prio.append(lambda: tail(0))
                prio.append(lambda: tail(1))

        def outproj_unit(b, qi, mq):
            t0 = b * S + qi * QB + mq * P
            ob = opool.tile([P, EMB], BF, tag="ob", name=f"ob_{b}_{qi}_{mq}")
            for half in range(2):
                ps = genps.tile(
                    [P, QB], F32, tag="gen", name=f"o_{b}_{qi}_{mq}_{half}"
                )
                nc.tensor.matmul(
                    ps[:],
                    attT[:, t0 : t0 + P],
                    WoT_sb[:, half * QB : (half + 1) * QB],
                    start=True, stop=True,
                )
                nc.vector.tensor_copy(ob[:, half * QB : (half + 1) * QB], ps[:])
            nc.gpsimd.dma_start(out[t0 : t0 + P, :], ob[:])

        # ---- main schedule: minimal serial head, everything else fillers ----
        qk_proj_unit(qhT, WqT_sb, "q", bqp_sb, 0, 0)
        for nb in range(NQ):
            qk_proj_unit(khT, WkT_sb, "k", bkp_sb, 0, nb)

        # filler queue: (min_step, fn).  min_step approximates when the
        # unit's input DMA has landed (1 step ~ 1.1us of attention).
        for m in range(SB):
            fillers.append((3 * (m // 4), lambda m=m: vh_proj_unit(0, m)))
        for nb in range(1, NQ):
            fillers.append(
                (9 + 2 * nb,
                 lambda nb=nb: qk_proj_unit(qhT, WqT_sb, "q", bqp_sb, 0, nb))
            )
        for nb in range(NQ):
            for f in qk_halves(qhT, WqT_sb, "q", bqp_sb, 1, nb):
                fillers.append((26, f))
        for nb in range(NQ):
            for f in qk_halves(khT, WkT_sb, "k", bkp_sb, 1, nb):
                fillers.append((37, f))
        for m in range(SB):
            fillers.append((50 + 3 * (m // 4), lambda m=m: vh_proj_unit(1, m)))

        # flat attention stream: 128 chunks; the score quad of chunk i+1 is
        # emitted ahead of att(i-1) and fillers so exp(i+1) is ready the
        # moment exp(i) retires (the PE refills one 2-bank score buffer
        # while ACT drains the other)
        chunks = [
            (b, qi, j) for b in range(B) for qi in range(NQ) for j in range(SB)
        ]
        NCH = len(chunks)
        att_tiles = {}

        def get_att(b, qi):
            if (b, qi) not in att_tiles:
                att_tiles[(b, qi)] = attps.tile(
                    [HD + 1, 2, QB], F32, tag="att", name=f"att_{b}_{qi}"
                )
            return att_tiles[(b, qi)]

        sc_pr = {}

        def attcall(i):
            b, qi, j = chunks[i]
            attAB = get_att(b, qi)
            att_chunk(b, j, sc_pr.pop(i), attAB)
            if j == SB - 1:
                normalize_block(b, qi, attAB, last=(i == NCH - 1))
                del att_tiles[(b, qi)]
                for mq in range(QB // P):
                    prio.append(lambda b=b, qi=qi, mq=mq: outproj_unit(b, qi, mq))

        sc_pr[0] = sc_chunk(*chunks[0])
        sc_pr[1] = sc_chunk(*chunks[1])
        fill(0)  # vhe chunk 0 before att(0)
        for i in range(1, NCH):
            if i + 1 < NCH:
                sc_pr[i + 1] = sc_chunk(*chunks[i + 1])
            attcall(i - 1)
            fill(i)
            if i <= 16:
                fill(i)
        attcall(NCH - 1)
        while prio:
            prio.popleft()()
        while fillers:
            fillers.popleft()[1]()

    nc.finalize()
    return nc


_NC_CACHE: dict = {}
_BO_CACHE: list = [None]


def _get_nc(with_bv: bool, with_bo: bool):
    key = (with_bv, with_bo)
    if key not in _NC_CACHE:
        _NC_CACHE[key] = _build_nc(*key)
    return _NC_CACHE[key]


def _feat_tiled(xT):
    """[EMB, n] -> [128, NE, n] contiguous (feature chunks on partitions)."""
    n = xT.shape[1]
    return np.ascontiguousarray(xT.reshape(NE, P, n).transpose(1, 0, 2))


def _stage(inputs):
    bf = ml_dtypes.bfloat16
    f32 = np.float32

    def arr(name):
        return np.asarray(inputs[name], f32)

    q, k, v = arr("q"), arr("k"), arr("v")
    Wq, Wk, Wv, Wo = arr("Wq"), arr("Wk"), arr("Wv"), arr("Wo")
    bq, bk, bv, bo = arr("bq"), arr("bk"), arr("bv"), arr("bo")

    with_bv = bool(np.any(bv))
    with_bo = bool(np.any(bo))
    _BO_CACHE[0] = bo if with_bo else None

    def xt(x3d):  # [B,S,EMB] -> [NBLK, 128, NE, QB] bf16, blocked contiguous
        xT = np.ascontiguousarray(x3d.reshape(T, EMB).T)  # [EMB, T]
        blocks = [
            _feat_tiled(xT[:, i * QB : (i + 1) * QB]) for i in range(NBLK)
        ]
        return np.ascontiguousarray(np.stack(blocks)).astype(bf)

    qTt, kTt, vTt = xt(q), xt(k), xt(v)

    in_maps = []
    for c in range(N_CORES):
        F = slice(c * P, (c + 1) * P)
        m = {
            "qT": qTt,
            "kT": kTt,
            "vT": vTt,
            "WqT": _feat_tiled(np.ascontiguousarray(Wq.T[:, F])).astype(bf),
            "WkT": _feat_tiled(np.ascontiguousarray(Wk.T[:, F])).astype(bf),
            "WvT": _feat_tiled(np.ascontiguousarray(Wv.T[:, F])).astype(bf),
            "WoT": np.ascontiguousarray(Wo.T[F, :]).astype(bf),
            "bqp": np.ascontiguousarray(bq[F][:, None]),
            "bkp": np.ascontiguousarray(bk[F][:, None]),
            "bvr": np.ascontiguousarray(bv[F][None, :]).astype(bf),
        }
        in_maps.append(m)
    return in_maps, with_bv, with_bo


def _assemble(results):
    acc = results[0]["out"].astype(np.float32)
    for c in range(1, N_CORES):
        acc += results[c]["out"].astype(np.float32)
    if _BO_CACHE[0] is not None:
        acc += _BO_CACHE[0]
    return acc.reshape(B, S, EMB)


def kernel(**inputs) -> np.ndarray:
    in_maps, with_bv, with_bo = _stage(inputs)
    nc = _get_nc(with_bv, with_bo)
    res = run_bass_kernel_spmd(nc, in_maps, list(range(N_CORES)))
    return _assemble(res.results)


# revision 3
# speedup vs baseline: 1.0065x; 1.0065x over previous
"""Multi-head attention (B=2, S=2048, EMB=1024, H=16) on 8 Trainium2 cores.

v3 — tensor-parallel over heads: core c owns heads {2c, 2c+1} (a 128-wide
feature slice F_c of the QKV projections and of Wo's rows).  Each core:
  - projects q/k/v for ALL 4096 tokens through its 1024x128 weight slices
  - runs attention for its 2 heads over both batches
  - computes the PARTIAL output projection out_c = att_c @ Wo[:, F_c].T
The 8 partial outputs (bf16) are summed on the host (linear in Wo rows), so
no device collectives are needed and no compute is redundant: per-core FLOPs
drop from ~15 G (query-sharded baseline with 4x-redundant K/V projections)
to ~8.6 G (perfect 1/8 split of total work).

Layouts (all device matmuls contract over the partition dim):
  - inputs staged HOST-side as [8 blocks, 128, 8, 512] so every x-block DMA
    is one 8 KiB-contiguous run per partition (descriptor count, not
    bandwidth, limited an earlier cut)
  - qhT/khT [128 = (headA 64 | headB 64), 4096 tok] bf16, feature-major
  - vhe [128 tok, chunk, head, 65] bf16 token-major with an all-ones 65th
    column so the att matmul accumulates softmax denominators for free
  - scores_T [k, q] per key-chunk via a 2x2 tile_position quad: 4 matmuls
    (2 heads x 2 key-halves, K=64 M=64 N=512) occupy disjoint PE-array
    quadrants and run concurrently (measured 54 ns/MM vs 217 serial)
  - exp: one ACT call per chunk ([128, 1024] over a 2-bank PSUM tile,
    scale=1/8, no max subtraction; |scores|/8 < ~7 so exp stays in range)
  - normalize: copy denom row, gpsimd partition_broadcast, DVE
    reciprocal_approx_fast on [64, 512] (full-width lanes), multiply
  - out partial [tok, 1024] bf16: one DMA per 128-token chunk, issued from
    the gpsimd queue to keep descriptor generation off the sync engine

Pipelining: the score PSUM is double-buffered (scps bufs=2), so the quad of
chunk j+1 fills one 2-bank buffer while ACT exp-drains the other — ACT runs
gapless and the PE never idles long enough for the HAM clock gate to drop
the array to 1.2 GHz (the dominant loss in earlier cuts).  All other PE work
(v-projection of batch 0, every projection of batch 1, the out-projection of
the previous qblock) is queued as <=1.2us filler units, popped one per chunk
into the PE's ~0.5us-per-chunk slack.
"""

from collections import deque
from contextlib import ExitStack

import numpy as np
import ml_dtypes

import concourse.bass as bass  # noqa: F401
import concourse.mybir as mybir
import concourse.tile as tile
from concourse import bacc
from concourse.bass_utils import run_bass_kernel_spmd

BF = mybir.dt.bfloat16
F32 = mybir.dt.float32

EMB = 1024
HEADS = 16
HD = EMB // HEADS          # 64
B, S = 2, 2048
N_CORES = 8
P = 128
NE = EMB // P              # 8 contraction chunks
T = B * S                  # 4096 tokens
NT = T // P                # 32 token chunks
SB = S // P                # 16 key chunks per batch
QB = 512                   # query block (psum bank width)
NQ = S // QB               # 4 qblocks per batch
NBLK = T // QB             # 8 x-blocks
EXPF = mybir.ActivationFunctionType.Exp
SCALE = 1.0 / np.sqrt(HD)  # 0.125


def _build_nc(with_bv: bool, with_bo: bool):
    nc = bacc.Bacc(num_devices=N_CORES)
    dp = nc.declare_dram_parameter
    qT = dp("qT", [NBLK, P, NE, QB], BF, isOutput=False)
    kT = dp("kT", [NBLK, P, NE, QB], BF, isOutput=False)
    vT = dp("vT", [NBLK, P, NE, QB], BF, isOutput=False)
    WqT = dp("WqT", [P, NE, P], BF, isOutput=False)
    WkT = dp("WkT", [P, NE, P], BF, isOutput=False)
    WvT = dp("WvT", [P, NE, P], BF, isOutput=False)
    WoT = dp("WoT", [P, EMB], BF, isOutput=False)
    bqp = dp("bqp", [P, 1], F32, isOutput=False)
    bkp = dp("bkp", [P, 1], F32, isOutput=False)
    bvr = dp("bvr", [1, P], BF, isOutput=False)
    out = dp("out", [T, EMB], BF, isOutput=True)

    with tile.TileContext(nc) as tc, ExitStack() as ctx:
        wpool = ctx.enter_context(tc.tile_pool(name="wts", bufs=1))
        apool = ctx.enter_context(tc.tile_pool(name="acts", bufs=1))
        xpool = ctx.enter_context(tc.tile_pool(name="xin", bufs=6))
        ppool = ctx.enter_context(tc.tile_pool(name="probs", bufs=4))
        dpool = ctx.enter_context(tc.tile_pool(name="dn", bufs=4))
        opool = ctx.enter_context(tc.tile_pool(name="ob", bufs=3))
        genps = ctx.enter_context(tc.tile_pool(name="gen", bufs=2, space="PSUM"))
        scps = ctx.enter_context(tc.tile_pool(name="sc", bufs=2, space="PSUM"))
        attps = ctx.enter_context(tc.tile_pool(name="att", bufs=1, space="PSUM"))

        qhT = apool.tile([P, T], BF, tag="qhT")
        khT = apool.tile([P, T], BF, tag="khT")
        vhe = apool.tile([P, NT, 2, HD + 1], BF, tag="vhe")
        attT = apool.tile([P, T], BF, tag="attT")

        WqT_sb = wpool.tile([P, NE, P], BF, tag="WqT")
        WkT_sb = wpool.tile([P, NE, P], BF, tag="WkT")
        WvT_sb = wpool.tile([P, NE, P], BF, tag="WvT")
        WoT_sb = wpool.tile([P, EMB], BF, tag="WoT")
        bqp_sb = wpool.tile([P, 1], F32, tag="bqp")
        bkp_sb = wpool.tile([P, 1], F32, tag="bkp")
        if with_bv:
            ones_b = wpool.tile([1, P], BF, tag="ones")
            nc.vector.memset(ones_b[:], 1.0)
            bvr_sb = wpool.tile([1, P], BF, tag="bvr")
            nc.sync.dma_start(bvr_sb[:], bvr[:])

        nc.vector.memset(vhe[:, :, :, HD], 1.0)

        # ---- DMA staging: one 8KiB-contiguous-per-partition DMA per block;
        # emitting all up front lets ring-slot WAR deps pace the prefetch ----
        nc.sync.dma_start(WqT_sb[:], WqT[:])
        nc.sync.dma_start(bqp_sb[:], bqp[:])
        nc.sync.dma_start(bkp_sb[:], bkp[:])
        nc.sync.dma_start(WkT_sb[:], WkT[:])
        nc.sync.dma_start(WvT_sb[:], WvT[:])
        xblocks = {}

        def stage_block(name, src, b, nb):
            xb = xpool.tile([P, NE, QB], BF, tag="xin", name=f"x_{name}_{b}_{nb}")
            nc.sync.dma_start(xb[:], src[b * NQ + nb, :, :, :])
            xblocks[(name, b, nb)] = xb

        # order: q-blk0 + all k of b0 (the minimum for the first score
        # quads), then v b0 (attention side), then q b0 rest, Wo, all of b1
        stage_block("q", qT, 0, 0)
        for nb in range(NQ):
            stage_block("k", kT, 0, nb)
        for nb in range(NQ):
            stage_block("v", vT, 0, nb)
        for nb in range(1, NQ):
            stage_block("q", qT, 0, nb)
        nc.sync.dma_start(WoT_sb[:], WoT[:])
        for name, src in (("q", qT), ("k", kT), ("v", vT)):
            for nb in range(NQ):
                stage_block(name, src, 1, nb)

        # ---- emission helpers ----
        def qk_proj_half(dst, W_sb, xname, bias_sb, b, nb, half, ps_box):
            """Half of a 512-token projection block (4 of 8 k-chunks)."""
            t0 = b * S + nb * QB
            xb = xblocks[(xname, b, nb)]
            if half == 0:
                ps_box[0] = genps.tile(
                    [P, QB], F32, tag="gen", name=f"pj_{xname}_{b}_{nb}"
                )
            ps = ps_box[0]
            for kk in range(half * 4, half * 4 + 4):
                nc.tensor.matmul(
                    ps[:],
                    W_sb[:, kk, :],
                    xb[:, kk, :],
                    start=(kk == 0),
                    stop=(kk == NE - 1),
                )
            if half == 1:
                nc.vector.tensor_scalar_add(
                    dst[:, t0 : t0 + QB], ps[:], bias_sb[:, 0:1]
                )

        def qk_proj_unit(dst, W_sb, xname, bias_sb, b, nb):
            box = [None]
            qk_proj_half(dst, W_sb, xname, bias_sb, b, nb, 0, box)
            qk_proj_half(dst, W_sb, xname, bias_sb, b, nb, 1, box)

        def qk_halves(dst, W_sb, xname, bias_sb, b, nb):
            box = [None]
            return [
                (lambda h=h: qk_proj_half(dst, W_sb, xname, bias_sb, b, nb, h, box))
                for h in range(2)
            ]

        def vh_proj_unit(b, m):
            """Project one 128-token chunk of v into vhe (token-major)."""
            mm = b * SB + m
            xb = xblocks[("v", b, m // (QB // P))]
            mi = m % (QB // P)
            ps = genps.tile([P, QB], F32, tag="gen", name=f"pv_{mm}")
            for kk in range(NE):
                nc.tensor.matmul(
                    ps[:, 0:P],
                    xb[:, kk, mi * P : (mi + 1) * P],
                    WvT_sb[:, kk, :],
                    start=(kk == 0),
                    stop=(kk == NE - 1) and not with_bv,
                )
            if with_bv:
                nc.tensor.matmul(
                    ps[:, 0:P], ones_b[:], bvr_sb[:], start=False, stop=True
                )
            nc.vector.tensor_copy(
                vhe[:, mm, :, 0:HD], ps[:, 0:P].rearrange("p (h d) -> p h d", d=HD)
            )

        prio: deque = deque()     # normalize tails + out-proj: no DMA deps
        fillers: deque = deque()  # (min_step, fn): gated on x-block DMA arrival
        last_prio = [-10]

        def fill(step):
            # prio units carry DVE-heavy epilogue work; at most one per 2
            # steps so the DVE never backlogs the gen-psum ring (a backlog
            # stalls the in-order PE queue and gaps the exp pipeline)
            if prio and step - last_prio[0] >= 2:
                last_prio[0] = step
                prio.popleft()()
            elif fillers and fillers[0][0] <= step:
                fillers.popleft()[1]()

        def sc_chunk(b, qi, j):
            """Scores+exp for one key chunk: a 2x2 matmul quad, one ACT call."""
            q0 = b * S + qi * QB
            j0 = (b * SB + j) * P
            sc = scps.tile([P, 2 * QB], F32, tag="sc", name=f"sc_{b}_{qi}_{j}")
            nc.tensor.matmul(
                sc[0:HD, 0:QB],
                khT[0:HD, j0 : j0 + HD],
                qhT[0:HD, q0 : q0 + QB],
                start=True, stop=True,
            )
            nc.tensor.matmul(
                sc[HD:P, 0:QB],
                khT[0:HD, j0 + HD : j0 + P],
                qhT[0:HD, q0 : q0 + QB],
                start=True, stop=True,
            )
            nc.tensor.matmul(
                sc[0:HD, QB:],
                khT[HD:P, j0 : j0 + HD],
                qhT[HD:P, q0 : q0 + QB],
                start=True, stop=True,
            )
            nc.tensor.matmul(
                sc[HD:P, QB:],
                khT[HD:P, j0 + HD : j0 + P],
                qhT[HD:P, q0 : q0 + QB],
                start=True, stop=True,
            )
            pr = ppool.tile([P, 2 * QB], BF, tag="pr", name=f"pr_{b}_{qi}_{j}")
            nc.scalar.activation(pr[:], sc[:], EXPF, scale=SCALE)
            return pr

        def att_chunk(b, j, pr, attAB):
            jb = b * SB + j
            first = j == 0
            last = j == SB - 1
            nc.tensor.matmul(
                attAB[:, 0, :], vhe[:, jb, 0, :], pr[:, 0:QB], start=first, stop=last
            )
            nc.tensor.matmul(
                attAB[:, 1, :], vhe[:, jb, 1, :], pr[:, QB:], start=first, stop=last
            )

        def normalize_block(b, qi, attAB, last=False):
            # copy the accumulators out of PSUM in ONE op: the att psum slot
            # is WAR-waited by the NEXT qblock's first att matmul (in-order
            # PE queue!), so it must free fast, not after the full
            # broadcast/reciprocal/multiply chain
            q0 = b * S + qi * QB
            if last:
                # nothing reuses the att psum slot afterwards: skip the
                # staging copy and read PSUM directly (shorter drain chain)
                au2 = attAB
            else:
                au2 = dpool.tile(
                    [HD + 1, 2, QB], F32, tag="au", name=f"au_{b}_{qi}"
                )
                nc.vector.tensor_copy(au2[:], attAB[:])

            def tail(h):
                d0 = dpool.tile([1, QB], F32, tag=f"d0{h}", name=f"d0_{b}_{qi}_{h}")
                nc.vector.tensor_copy(d0[:], au2[HD : HD + 1, h, :])
                r0 = dpool.tile([1, QB], F32, tag=f"r0{h}", name=f"r0_{b}_{qi}_{h}")
                nc.vector.reciprocal_approx_fast(r0[:], d0[:])
                rb = dpool.tile([HD, QB], F32, tag=f"rb{h}", name=f"rb_{b}_{qi}_{h}")
                nc.gpsimd.partition_broadcast(rb[:], r0[:])
                nc.vector.tensor_mul(
                    attT[h * HD : (h + 1) * HD, q0 : q0 + QB],
                    au2[0:HD, h, :],
                    rb[:],
                )

            if last:
                tail(0)
                tail(1)
            else:
                prio.append(lambda: tail(0))
                prio.append(lambda: tail(1))

        def outproj_unit(b, qi, mq):
            t0 = b * S + qi * QB + mq * P
            ob = opool.tile([P, EMB], BF, tag="ob", name=f"ob_{b}_{qi}_{mq}")
            for half in range(2):
                ps = genps.tile(
                    [P, QB], F32, tag="gen", name=f"o_{b}_{qi}_{mq}_{half}"
                )
                nc.tensor.matmul(
                    ps[:],
                    attT[:, t0 : t0 + P],
                    WoT_sb[:, half * QB : (half + 1) * QB],
                    start=True, stop=True,
                )
                nc.vector.tensor_copy(ob[:, half * QB : (half + 1) * QB], ps[:])
            nc.gpsimd.dma_start(out[t0 : t0 + P, :], ob[:])

        # ---- main schedule: minimal serial head, everything else fillers ----
        qk_proj_unit(qhT, WqT_sb, "q", bqp_sb, 0, 0)
        for nb in range(NQ):
            qk_proj_unit(khT, WkT_sb, "k", bkp_sb, 0, nb)

        # filler queue: (min_step, fn).  min_step approximates when the
        # unit's input DMA has landed (1 step ~ 1.1us of attention).
        for m in range(SB):
            fillers.append((3 * (m // 4), lambda m=m: vh_proj_unit(0, m)))
        for nb in range(1, NQ):
            fillers.append(
                (9 + 2 * nb,
                 lambda nb=nb: qk_proj_unit(qhT, WqT_sb, "q", bqp_sb, 0, nb))
            )
        for nb in range(NQ):
            for f in qk_halves(qhT, WqT_sb, "q", bqp_sb, 1, nb):
                fillers.append((26, f))
        for nb in range(NQ):
            for f in qk_halves(khT, WkT_sb, "k", bkp_sb, 1, nb):
                fillers.append((37, f))
        for m in range(SB):
            fillers.append((50 + 3 * (m // 4), lambda m=m: vh_proj_unit(1, m)))

        # flat attention stream: 128 chunks; the score quad of chunk i+1 is
        # emitted ahead of att(i-1) and fillers so exp(i+1) is ready the
        # moment exp(i) retires (the PE refills one 2-bank score buffer
        # while ACT drains the other)
        chunks = [
            (b, qi, j) for b in range(B) for qi in range(NQ) for j in range(SB)
        ]
        NCH = len(chunks)
        att_tiles = {}

        def get_att(b, qi):
            if (b, qi) not in att_tiles:
                att_tiles[(b, qi)] = attps.tile(
                    [HD + 1, 2, QB], F32, tag="att", name=f"att_{b}_{qi}"
                )
            return att_tiles[(b, qi)]

        sc_pr = {}

        def att_half(i, h):
            # one head's att matmul per step: halves the per-step PE load on
            # the critical chain so it fits the exp period even at cold clock
            b, qi, j = chunks[i]
            attAB = get_att(b, qi)
            pr = sc_pr[i]
            nc.tensor.matmul(
                attAB[:, h, :],
                vhe[:, b * SB + j, h, :],
                pr[:, h * QB : (h + 1) * QB],
                start=(j == 0),
                stop=(j == SB - 1),
            )
            if h == 1:
                sc_pr.pop(i)
                if j == SB - 1:
                    normalize_block(b, qi, attAB, last=(i == NCH - 1))
                    del att_tiles[(b, qi)]
                    for mq in range(QB // P):
                        prio.append(
                            lambda b=b, qi=qi, mq=mq: outproj_unit(b, qi, mq)
                        )

        sc_pr[0] = sc_chunk(*chunks[0])
        sc_pr[1] = sc_chunk(*chunks[1])
        fill(0)  # vhe chunk 0 before att(0)
        for i in range(1, NCH):
            if i + 1 < NCH:
                sc_pr[i + 1] = sc_chunk(*chunks[i + 1])
            if i >= 2:
                att_half(i - 2, 1)   # older chunk's head B first (ordering!)
            att_half(i - 1, 0)
            fill(i)
            if i <= 16:
                fill(i)
        att_half(NCH - 2, 1)
        att_half(NCH - 1, 0)
        att_half(NCH - 1, 1)
        while prio:
            prio.popleft()()
        while fillers:
            fillers.popleft()[1]()

    nc.finalize()
    return nc


_NC_CACHE: dict = {}
_BO_CACHE: list = [None]


def _get_nc(with_bv: bool, with_bo: bool):
    key = (with_bv, with_bo)
    if key not in _NC_CACHE:
        _NC_CACHE[key] = _build_nc(*key)
    return _NC_CACHE[key]


def _feat_tiled(xT):
    """[EMB, n] -> [128, NE, n] contiguous (feature chunks on partitions)."""
    n = xT.shape[1]
    return np.ascontiguousarray(xT.reshape(NE, P, n).transpose(1, 0, 2))


def _stage(inputs):
    bf = ml_dtypes.bfloat16
    f32 = np.float32

    def arr(name):
        return np.asarray(inputs[name], f32)

    q, k, v = arr("q"), arr("k"), arr("v")
    Wq, Wk, Wv, Wo = arr("Wq"), arr("Wk"), arr("Wv"), arr("Wo")
    bq, bk, bv, bo = arr("bq"), arr("bk"), arr("bv"), arr("bo")

    with_bv = bool(np.any(bv))
    with_bo = bool(np.any(bo))
    _BO_CACHE[0] = bo if with_bo else None

    def xt(x3d):  # [B,S,EMB] -> [NBLK, 128, NE, QB] bf16, blocked contiguous
        xT = np.ascontiguousarray(x3d.reshape(T, EMB).T)  # [EMB, T]
        blocks = [
            _feat_tiled(xT[:, i * QB : (i + 1) * QB]) for i in range(NBLK)
        ]
        return np.ascontiguousarray(np.stack(blocks)).astype(bf)

    qTt, kTt, vTt = xt(q), xt(k), xt(v)

    in_maps = []
    for c in range(N_CORES):
        F = slice(c * P, (c + 1) * P)
        m = {
            "qT": qTt,
            "kT": kTt,
            "vT": vTt,
            "WqT": _feat_tiled(np.ascontiguousarray(Wq.T[:, F])).astype(bf),
            "WkT": _feat_tiled(np.ascontiguousarray(Wk.T[:, F])).astype(bf),
            "WvT": _feat_tiled(np.ascontiguousarray(Wv.T[:, F])).astype(bf),
            "WoT": np.ascontiguousarray(Wo.T[F, :]).astype(bf),
            "bqp": np.ascontiguousarray(bq[F][:, None]),
            "bkp": np.ascontiguousarray(bk[F][:, None]),
            "bvr": np.ascontiguousarray(bv[F][None, :]).astype(bf),
        }
        in_maps.append(m)
    return in_maps, with_bv, with_bo


def _assemble(results):
    acc = results[0]["out"].astype(np.float32)
    for c in range(1, N_CORES):
        acc += results[c]["out"].astype(np.float32)
    if _BO_CACHE[0] is not None:
        acc += _BO_CACHE[0]
    return acc.reshape(B, S, EMB)


def kernel(**inputs) -> np.ndarray:
    in_maps, with_bv, with_bo = _stage(inputs)
    nc = _get_nc(with_bv, with_bo)
    res = run_bass_kernel_spmd(nc, in_maps, list(range(N_CORES)))
    return _assemble(res.results)
